# Initial kernel scaffold
#
"""MultiHeadAttention (B=2, S=2048, D=1024, H=16) on 8 trn2 cores.

Tensor-parallel over heads: core c owns heads 2c, 2c+1 (128 output features).
Per core:
  phase A: project q/k/v transposed:  qT = (Wq_c/8) @ X^T   [128 feat, 4096 tok]
           (X^T streamed from DRAM in bf16; W shards resident)
           v additionally PE-transposed to natural layout and augmented with a
           ones column per 128-token chunk (rowsum rides the attn@v matmul).
  phase B: per (batch, head):  S^T = kT^T-chunks @ qT  (scores transposed:
           key-tokens on partitions, query-tokens free)  ->  exp on ScalarE
           -> multiply by notmask (bf16, DVE) -> out^T[65, n] += v_aug^T @ expT
           accumulated over key chunks in PSUM.  out^T row 64 is the softmax
           denominator.  Division happens on host in fp32.
"""

import sys

sys.path.insert(0, "/opt/trn_rl_repo")

import numpy as np
import ml_dtypes

import concourse.mybir as mybir
import concourse.tile as tile
from concourse import bacc
from concourse.bass_utils import run_bass_kernel_spmd
from concourse.masks import make_identity

BF16 = mybir.dt.bfloat16
F32 = mybir.dt.float32
NP_BF16 = ml_dtypes.bfloat16

NCORES = 8
B, S, D = 2, 2048, 1024
H, DH = 16, 64
HPC = H // NCORES  # heads per core = 2
MPC = HPC * DH  # output features per core = 128
T = B * S  # 4096 tokens
NKC = D // 128  # 8 contraction chunks for projections
NNC = T // 512  # 8 token chunks of 512 (projection N tiling)
NJC = S // 128  # 16 key-token chunks per batch
NTC = T // 128  # 32 global token chunks (v_aug)
VW = DH + 1  # 65: head dim + ones column

_CACHE: dict = {}


def _emit(nc, dins, dout):
    from contextlib import ExitStack

    tc = dins["_tc"]
    with ExitStack() as ctx:
        singles = ctx.enter_context(tc.tile_pool(name="singles", bufs=1))

        w_sb, b_sb = {}, {}
        for t in ("q", "k", "v"):
            w = singles.tile([128, NKC, 128], BF16, tag=f"w{t}")
            nc.sync.dma_start(
                out=w,
                in_=dins[f"w{t}T"].ap().rearrange("(c p) m -> p c m", p=128),
            )
            w_sb[t] = w
            bb = singles.tile([128, 1], F32, tag=f"b{t}")
            nc.sync.dma_start(
                out=bb, in_=dins[f"b{t}"].ap().rearrange("(p o) -> p o", o=1)
            )
            b_sb[t] = bb

        qT = singles.tile([128, T], BF16, tag="qT")
        kT = singles.tile([128, T], BF16, tag="kT")
        vT = singles.tile([128, T], BF16, tag="vT")
        projT = {"q": qT, "k": kT, "v": vT}

        v_aug = [
            singles.tile([128, NTC * VW], BF16, tag=f"vaug{h}") for h in range(HPC)
        ]
        for h in range(HPC):
            nc.vector.memset(v_aug[h], 1.0)

        ident = singles.tile([128, 128], BF16, tag="ident")
        make_identity(nc, ident)

        # ---- phase A: projections (+ v transpose) ----
        with (
            tc.tile_pool(name="xpanels", bufs=2) as xp,
            tc.tile_pool(name="psA", bufs=2, space="PSUM") as psA,
            tc.tile_pool(name="psT", bufs=2, space="PSUM") as psT,
        ):
            for t in ("v", "k", "q"):
                for ncb in range(NNC):
                    xtile = xp.tile([128, NKC, 512], BF16, tag=f"x{t}")
                    nc.sync.dma_start(
                        out=xtile,
                        in_=dins[f"x{t}T"]
                        .ap()[:, ncb * 512 : (ncb + 1) * 512]
                        .rearrange("(c p) n -> p c n", p=128),
                    )
                    ps = psA.tile([128, 512], F32, tag="proj")
                    for kc in range(NKC):
                        nc.tensor.matmul(
                            ps,
                            lhsT=w_sb[t][:, kc, :],
                            rhs=xtile[:, kc, :],
                            start=(kc == 0),
                            stop=(kc == NKC - 1),
                        )
                    nc.vector.tensor_scalar_add(
                        out=projT[t][:, ncb * 512 : (ncb + 1) * 512],
                        in0=ps,
                        scalar1=b_sb[t],
                    )
            for tch in range(NTC):
                pst = psT.tile([128, 128], F32, tag="vt")
                nc.tensor.transpose(
                    out=pst, in_=vT[:, tch * 128 : (tch + 1) * 128], identity=ident
                )
                for h in range(HPC):
                    nc.vector.tensor_copy(
                        out=v_aug[h][:, tch * VW : tch * VW + DH],
                        in_=pst[:, h * DH : (h + 1) * DH],
                    )

        # ---- phase B: attention ----
        with (
            tc.tile_pool(name="nmp", bufs=4) as nmp,
            tc.tile_pool(name="expp", bufs=3) as expp,
            tc.tile_pool(name="outsb", bufs=2) as outsb,
            tc.tile_pool(name="psS", bufs=2, space="PSUM") as psS,
            tc.tile_pool(name="psO", bufs=1, space="PSUM") as psO,
        ):
            for b in range(B):
                for nh in range(2):
                    nbase = b * S + nh * 1024
                    outps = [
                        psO.tile([VW, 1024], F32, tag=f"out{h}") for h in range(HPC)
                    ]
                    for jc in range(NJC):
                        nm = nmp.tile([128, 1024], BF16, tag="nm")
                        nc.sync.dma_start(
                            out=nm,
                            in_=dins["nmT"].ap()[
                                b,
                                jc * 128 : (jc + 1) * 128,
                                nh * 1024 : (nh + 1) * 1024,
                            ],
                        )
                        tglob = b * NJC + jc
                        for h in range(HPC):
                            ps = psS.tile([128, 1024], F32, tag="scores")
                            for s2 in range(2):
                                nc.tensor.matmul(
                                    ps[:, s2 * 512 : (s2 + 1) * 512],
                                    lhsT=kT[
                                        h * DH : (h + 1) * DH,
                                        tglob * 128 : (tglob + 1) * 128,
                                    ],
                                    rhs=qT[
                                        h * DH : (h + 1) * DH,
                                        nbase + s2 * 512 : nbase + (s2 + 1) * 512,
                                    ],
                                    start=True,
                                    stop=True,
                                )
                            et = expp.tile([128, 1024], BF16, tag="exp")
                            nc.scalar.activation(
                                out=et,
                                in_=ps,
                                func=mybir.ActivationFunctionType.Exp,
                            )
                            nc.vector.tensor_mul(et, et, nm)
                            for s2 in range(2):
                                nc.tensor.matmul(
                                    outps[h][:, s2 * 512 : (s2 + 1) * 512],
                                    lhsT=v_aug[h][:, tglob * VW : tglob * VW + VW],
                                    rhs=et[:, s2 * 512 : (s2 + 1) * 512],
                                    start=(jc == 0),
                                    stop=(jc == NJC - 1),
                                )
                    for h in range(HPC):
                        osb = outsb.tile([VW, 1024], F32, tag="osb")
                        nc.vector.tensor_copy(out=osb, in_=outps[h])
                        nc.sync.dma_start(
                            out=dout.ap()[b, h, :, nh * 1024 : (nh + 1) * 1024],
                            in_=osb,
                        )


def _build():
    if "nc" in _CACHE:
        return _CACHE["nc"]
    nc = bacc.Bacc("TRN2", target_bir_lowering=False, debug=False)
    dins = {}
    for t in ("q", "k", "v"):
        dins[f"x{t}T"] = nc.dram_tensor(f"x{t}T", [D, T], BF16, kind="ExternalInput")
        dins[f"w{t}T"] = nc.dram_tensor(f"w{t}T", [D, MPC], BF16, kind="ExternalInput")
        dins[f"b{t}"] = nc.dram_tensor(f"b{t}", [MPC], F32, kind="ExternalInput")
    dins["nmT"] = nc.dram_tensor("nmT", [B, S, S], BF16, kind="ExternalInput")
    dout = nc.dram_tensor("out", [B, HPC, VW, S], F32, kind="ExternalOutput")

    with tile.TileContext(nc) as tc:
        dins["_tc"] = tc
        _emit(nc, dins, dout)
        del dins["_tc"]
    nc.compile()
    _CACHE["nc"] = nc
    return nc


def _prep_inputs(query, key, value, mask, Wq, bq, Wk, bk, Wv, bv):
    """Host-side shard prep. Returns per-core input maps."""
    xs = {}
    for name, x in (("q", query), ("k", key), ("v", value)):
        xt = np.ascontiguousarray(
            np.asarray(x, dtype=np.float32).reshape(T, D).T
        ).astype(NP_BF16)
        xs[f"x{name}T"] = xt

    nm = (~np.asarray(mask)).astype(NP_BF16)
    nmT = np.ascontiguousarray(np.transpose(nm, (0, 2, 1)))

    Wq = np.asarray(Wq, dtype=np.float32)
    Wk = np.asarray(Wk, dtype=np.float32)
    Wv = np.asarray(Wv, dtype=np.float32)
    bq = np.asarray(bq, dtype=np.float32)
    bk = np.asarray(bk, dtype=np.float32)
    bv = np.asarray(bv, dtype=np.float32)
    scale = 1.0 / np.sqrt(np.float32(DH))

    in_maps = []
    for c in range(NCORES):
        r = slice(c * MPC, (c + 1) * MPC)
        m = dict(xs)
        m["nmT"] = nmT
        m["wqT"] = np.ascontiguousarray((Wq[r] * scale).T).astype(NP_BF16)
        m["wkT"] = np.ascontiguousarray(Wk[r].T).astype(NP_BF16)
        m["wvT"] = np.ascontiguousarray(Wv[r].T).astype(NP_BF16)
        m["bq"] = np.ascontiguousarray(bq[r] * scale)
        m["bk"] = np.ascontiguousarray(bk[r])
        m["bv"] = np.ascontiguousarray(bv[r])
        in_maps.append(m)
    return in_maps


def _assemble(results):
    """results: per-core dicts with 'out' [B, HPC, 65, S] f32 -> [B, S, D]."""
    full = np.empty((B, S, D), dtype=np.float32)
    for c in range(NCORES):
        o = results[c]["out"]
        for b in range(B):
            for h in range(HPC):
                num = o[b, h, :DH, :]  # [64, S]
                den = o[b, h, DH, :]  # [S]
                col = c * MPC + h * DH
                full[b, :, col : col + DH] = (num / den).T
    return full


def kernel(query, key, value, mask, Wq, bq, Wk, bk, Wv, bv, **extra):
    nc = _build()
    in_maps = _prep_inputs(query, key, value, mask, Wq, bq, Wk, bk, Wv, bv)
    res = run_bass_kernel_spmd(nc, in_maps, core_ids=list(range(NCORES)))
    return _assemble(res.results)


def run_traced(inputs, **trace_kwargs):
    """For test.py: run with NTFF tracing, return (output, BassKernelResults)."""
    nc = _build()
    in_maps = _prep_inputs(**{k: inputs[k] for k in (
        "query", "key", "value", "mask", "Wq", "bq", "Wk", "bk", "Wv", "bv")})
    res = run_bass_kernel_spmd(
        nc, in_maps, core_ids=list(range(NCORES)), trace=True, **trace_kwargs
    )
    return _assemble(res.results), res


# revision 11
# speedup vs baseline: 1.2689x; 1.2689x over previous
"""MultiHeadAttention (B=2, S=2048, D=1024, H=16) on 8 trn2 cores.

Tensor-parallel over heads: core c owns heads 2c, 2c+1 (128 output features).
Per core:
  phase A: project q/k/v transposed:  qT = (Wq_c/8) @ X^T   [128 feat, 4096 tok]
           (X^T streamed from DRAM in bf16; W shards resident)
           v additionally PE-transposed to natural layout and augmented with a
           ones column per 128-token chunk (rowsum rides the attn@v matmul).
  phase B: per (batch, head):  S^T = kT^T-chunks @ qT  (scores transposed:
           key-tokens on partitions, query-tokens free)  ->  exp on ScalarE
           -> multiply by notmask (bf16, DVE) -> out^T[65, n] += v_aug^T @ expT
           accumulated over key chunks in PSUM.  out^T row 64 is the softmax
           denominator.  Division happens on host in fp32.
"""

import sys

sys.path.insert(0, "/opt/trn_rl_repo")

import numpy as np
import ml_dtypes

import concourse.mybir as mybir
import concourse.tile as tile
from concourse import bacc
from concourse.bass_utils import run_bass_kernel_spmd
from concourse.masks import make_identity

BF16 = mybir.dt.bfloat16
F32 = mybir.dt.float32
NP_BF16 = ml_dtypes.bfloat16

NCORES = 8
B, S, D = 2, 2048, 1024
H, DH = 16, 64
HPC = H // NCORES  # heads per core = 2
MPC = HPC * DH  # output features per core = 128
T = B * S  # 4096 tokens
NKC = D // 128  # 8 contraction chunks for projections
NNC = T // 512  # 8 token chunks of 512 (projection N tiling)
NJC = S // 128  # 16 key-token chunks per batch
NTC = T // 128  # 32 global token chunks (v_aug)
VW = DH + 1  # 65: head dim + ones column

_CACHE: dict = {}


def _emit(nc, dins, dout):
    from contextlib import ExitStack

    tc = dins["_tc"]
    NCB = S // 512  # 4 token chunks of 512 per batch
    with ExitStack() as ctx:
        singles = ctx.enter_context(tc.tile_pool(name="singles", bufs=1))

        w_sb, b_sb = {}, {}
        for t in ("q", "k", "v"):
            w = singles.tile([128, NKC, 128], BF16, tag=f"w{t}")
            nc.sync.dma_start(
                out=w,
                in_=dins[f"w{t}T"].ap().rearrange("(c p) m -> p c m", p=128),
            )
            w_sb[t] = w
            bb = singles.tile([128, 1], F32, tag=f"b{t}")
            nc.sync.dma_start(
                out=bb, in_=dins[f"b{t}"].ap().rearrange("(p o) -> p o", o=1)
            )
            b_sb[t] = bb

        qT = singles.tile([128, T], BF16, tag="qT")
        kT = singles.tile([128, T], BF16, tag="kT")
        vT = singles.tile([128, T], BF16, tag="vT")
        projT = {"q": qT, "k": kT, "v": vT}

        v_aug = [
            singles.tile([128, NTC * VW], BF16, tag=f"vaug{h}", name=f"vaug{h}")
            for h in range(HPC)
        ]
        for h in range(HPC):
            nc.vector.memset(v_aug[h], 1.0)

        ident = singles.tile([128, 128], BF16, tag="ident")
        make_identity(nc, ident)

        # batch-resident transposed notmask, one tile per batch
        nm_sb = [
            singles.tile([128, NJC, S], BF16, tag=f"nm{b}", name=f"nm{b}")
            for b in range(B)
        ]

        xp = ctx.enter_context(tc.tile_pool(name="xpanels", bufs=3))
        expp = ctx.enter_context(tc.tile_pool(name="expp", bufs=3))
        outsb = ctx.enter_context(tc.tile_pool(name="outsb", bufs=2))
        # PSUM budget (8 banks total): proj/vt shared tag 2, scores 4, out 2
        psA = ctx.enter_context(tc.tile_pool(name="psA", bufs=2, space="PSUM"))
        psS = ctx.enter_context(tc.tile_pool(name="psS", bufs=2, space="PSUM"))
        psO = ctx.enter_context(tc.tile_pool(name="psO", bufs=1, space="PSUM"))

        def proj_chunk(t, b, ncb):
            """Project one 512-token chunk of tensor t: X^T panel -> projT."""
            col = b * S + ncb * 512
            xtile = xp.tile([128, NKC, 512], BF16, tag="xpanel", name="xpanel")
            nc.sync.dma_start(
                out=xtile,
                in_=dins[f"x{t}T"]
                .ap()[:, col : col + 512]
                .rearrange("(c p) n -> p c n", p=128),
            )
            ps = psA.tile([128, 512], F32, tag="proj")
            for kc in range(NKC):
                nc.tensor.matmul(
                    ps,
                    lhsT=w_sb[t][:, kc, :],
                    rhs=xtile[:, kc, :],
                    start=(kc == 0),
                    stop=(kc == NKC - 1),
                )
            # drain on ScalarE (idle during projections); bias add rides along
            nc.scalar.activation(
                out=projT[t][:, col : col + 512],
                in_=ps,
                func=mybir.ActivationFunctionType.Identity,
                bias=b_sb[t],
            )

        def v_transpose(b, ncb):
            """PE-transpose 512 projected v columns into v_aug (4 chunks)."""
            tbase = b * NJC + ncb * 4
            pst = psA.tile([128, 512], BF16, tag="proj", name="vtps")
            for i in range(4):
                nc.tensor.transpose(
                    out=pst[:, i * 128 : (i + 1) * 128],
                    in_=vT[:, (tbase + i) * 128 : (tbase + i + 1) * 128],
                    identity=ident,
                )
            for h in range(HPC):
                # strided copy: 4 chunks x 64 head cols -> v_aug stride-65 slots
                src = pst.rearrange("p (i d) -> p i d", i=4)[:, :, h * DH : (h + 1) * DH]
                dst = v_aug[h][:, tbase * VW : (tbase + 4) * VW].rearrange(
                    "p (i w) -> p i w", i=4
                )[:, :, 0:DH]
                nc.vector.tensor_copy(out=dst, in_=src)

        def emit_proj(b):
            for ncb in range(NCB):
                proj_chunk("k", b, ncb)
                proj_chunk("v", b, ncb)
                v_transpose(b, ncb)
                proj_chunk("q", b, ncb)

        def emit_mask_dma(b):
            for jc in range(NJC):
                nc.sync.dma_start(
                    out=nm_sb[b][:, jc, :],
                    in_=dins["nmT"].ap()[b, jc * 128 : (jc + 1) * 128, :],
                )

        def emit_attn(b):
            for nh in range(2):
                nbase = b * S + nh * 1024
                for h in range(HPC):
                    outps = psO.tile([VW, 1024], F32, tag="out", name="outps")
                    for jc in range(NJC):
                        tglob = b * NJC + jc
                        ps = psS.tile([128, 1024], F32, tag="scores")
                        for s2 in range(2):
                            nc.tensor.matmul(
                                ps[:, s2 * 512 : (s2 + 1) * 512],
                                lhsT=kT[
                                    h * DH : (h + 1) * DH,
                                    tglob * 128 : (tglob + 1) * 128,
                                ],
                                rhs=qT[
                                    h * DH : (h + 1) * DH,
                                    nbase + s2 * 512 : nbase + (s2 + 1) * 512,
                                ],
                                start=True,
                                stop=True,
                            )
                        et = expp.tile([128, 1024], BF16, tag="exp")
                        nc.scalar.activation(
                            out=et, in_=ps, func=mybir.ActivationFunctionType.Exp
                        )
                        nc.vector.tensor_mul(
                            et, et, nm_sb[b][:, jc, nh * 1024 : (nh + 1) * 1024]
                        )
                        for s2 in range(2):
                            nc.tensor.matmul(
                                outps[:, s2 * 512 : (s2 + 1) * 512],
                                lhsT=v_aug[h][:, tglob * VW : tglob * VW + VW],
                                rhs=et[:, s2 * 512 : (s2 + 1) * 512],
                                start=(jc == 0),
                                stop=(jc == NJC - 1),
                            )
                    osb = outsb.tile([VW, 1024], F32, tag="osb")
                    nc.vector.tensor_copy(out=osb, in_=outps)
                    nc.sync.dma_start(
                        out=dout.ap()[b, h, :, nh * 1024 : (nh + 1) * 1024],
                        in_=osb,
                    )

        for _ in range(dins.get("_repeat", 1)):
            emit_proj(0)
            emit_mask_dma(0)
            emit_proj(1)
            emit_attn(0)
            emit_mask_dma(1)
            emit_attn(1)


def _build(repeat=1):
    key = ("nc", repeat)
    if key in _CACHE:
        return _CACHE[key]
    nc = bacc.Bacc("TRN2", target_bir_lowering=False, debug=False)
    dins = {}
    for t in ("q", "k", "v"):
        dins[f"x{t}T"] = nc.dram_tensor(f"x{t}T", [D, T], BF16, kind="ExternalInput")
        dins[f"w{t}T"] = nc.dram_tensor(f"w{t}T", [D, MPC], BF16, kind="ExternalInput")
        dins[f"b{t}"] = nc.dram_tensor(f"b{t}", [MPC], F32, kind="ExternalInput")
    dins["nmT"] = nc.dram_tensor("nmT", [B, S, S], BF16, kind="ExternalInput")
    dout = nc.dram_tensor("out", [B, HPC, VW, S], F32, kind="ExternalOutput")

    with tile.TileContext(nc) as tc:
        dins["_tc"] = tc
        dins["_repeat"] = repeat
        _emit(nc, dins, dout)
        del dins["_tc"], dins["_repeat"]
    nc.compile()
    _CACHE[key] = nc
    return nc


def _prep_inputs(query, key, value, mask, Wq, bq, Wk, bk, Wv, bv):
    """Host-side shard prep. Returns per-core input maps."""
    xs = {}
    for name, x in (("q", query), ("k", key), ("v", value)):
        xt = np.ascontiguousarray(
            np.asarray(x, dtype=np.float32).reshape(T, D).T
        ).astype(NP_BF16)
        xs[f"x{name}T"] = xt

    nm = (~np.asarray(mask)).astype(NP_BF16)
    nmT = np.ascontiguousarray(np.transpose(nm, (0, 2, 1)))

    Wq = np.asarray(Wq, dtype=np.float32)
    Wk = np.asarray(Wk, dtype=np.float32)
    Wv = np.asarray(Wv, dtype=np.float32)
    bq = np.asarray(bq, dtype=np.float32)
    bk = np.asarray(bk, dtype=np.float32)
    bv = np.asarray(bv, dtype=np.float32)
    scale = 1.0 / np.sqrt(np.float32(DH))

    in_maps = []
    for c in range(NCORES):
        r = slice(c * MPC, (c + 1) * MPC)
        m = dict(xs)
        m["nmT"] = nmT
        m["wqT"] = np.ascontiguousarray((Wq[r] * scale).T).astype(NP_BF16)
        m["wkT"] = np.ascontiguousarray(Wk[r].T).astype(NP_BF16)
        m["wvT"] = np.ascontiguousarray(Wv[r].T).astype(NP_BF16)
        m["bq"] = np.ascontiguousarray(bq[r] * scale)
        m["bk"] = np.ascontiguousarray(bk[r])
        m["bv"] = np.ascontiguousarray(bv[r])
        in_maps.append(m)
    return in_maps


def _assemble(results):
    """results: per-core dicts with 'out' [B, HPC, 65, S] f32 -> [B, S, D]."""
    full = np.empty((B, S, D), dtype=np.float32)
    for c in range(NCORES):
        o = results[c]["out"]
        for b in range(B):
            for h in range(HPC):
                num = o[b, h, :DH, :]  # [64, S]
                den = o[b, h, DH, :]  # [S]
                col = c * MPC + h * DH
                full[b, :, col : col + DH] = (num / den).T
    return full


def kernel(query, key, value, mask, Wq, bq, Wk, bk, Wv, bv, **extra):
    nc = _build()
    in_maps = _prep_inputs(query, key, value, mask, Wq, bq, Wk, bk, Wv, bv)
    res = run_bass_kernel_spmd(nc, in_maps, core_ids=list(range(NCORES)))
    return _assemble(res.results)


def run_traced(inputs, **trace_kwargs):
    """For test.py: run with NTFF tracing, return (output, BassKernelResults)."""
    nc = _build()
    in_maps = _prep_inputs(**{k: inputs[k] for k in (
        "query", "key", "value", "mask", "Wq", "bq", "Wk", "bk", "Wv", "bv")})
    try:
        res = run_bass_kernel_spmd(
            nc, in_maps, core_ids=list(range(NCORES)), trace=True, **trace_kwargs
        )
    except ModuleNotFoundError:
        res = run_bass_kernel_spmd(nc, in_maps, core_ids=list(range(NCORES)))
    return _assemble(res.results), res
